# revision 13
# baseline (speedup 1.0000x reference)
"""Trainium2 Bass kernel for nn_Attn: out = softmax(v . (W @ q_s + b)) over s.

Key algebraic identity:
    energies[s] = v . (W @ q[s] + b) = q[s] . (W^T v) + (v . b)
The (v . b) term is constant across s and softmax is shift-invariant, so it
drops out. The kernel computes u = W^T v (tiny), then a matvec
energies = question @ u, then a global softmax. This is memory-bound:
question (128 MiB fp32) must stream from HBM once; everything else is noise.

Distribution over 8 NeuronCores — sequence (s) sharding:
  - core r owns tokens [r*4096, (r+1)*4096); host sends the transposed slab
    QT[:, r*4096:(r+1)*4096] reshaped [128, 8, 4096] so chunk c holds
    h = p*8 + c on partition p (contiguous 16 KB descriptors)
  - W is REPLICATED with v folded in as column 0 of each chunk row
    (W_aug [128, 8, 1025], 32 KB descriptors): ncfw collectives have a
    ~40-60 us per-kernel setup floor, so an early u-AllGather can never
    beat just streaming 4 extra MB of W
  - the 21 MB stream is split across BOTH HWDGE queues (sync: W,q0,q1,q2;
    scalar: q3..q7) — one queue saturates ~340 GB/s, two reach ~420 —
    and the energy chunk loop consumes in expected arrival order
  - u = W^T v via 16 M=1 matmuls, then 8 tiny transposes build
    u_sb[p, c] = u[p*8+c]
  - energy matmuls contract h chunk-by-chunk, accumulating in PSUM
    (start/stop); four M=1 fp32 matmuls pack into the four 32-column PE
    groups (tile_position, psum base partitions 0/32/64/96)
  - energies land in e_all[:, 0:32] (token t at partition t//32, col t%32)
    and are exchanged via 7 remote_dma_broadcast sends (direct SDMA
    SBUF->SBUF into the peers' e_all, XOR slot addressing: slot k holds
    tokens of core self^k) — bypassing the ncfw collective data path
    (~3 us instead of ~19). A tiny 4-byte ncfw AllGather (overlapped with
    the stream) proves every core entered the kernel before bytes fly.
    The manual-semaphore handshake lives inside tc.tile_critical(), which
    the tile scheduler treats as an opaque unit (its no-exec sim cannot
    model remotely-incremented semaphores). Softmax stats over [128, 256]
    are permutation-invariant; each core writes out ONLY its own slot-0
    slice [4096] and the host concatenates by rank, so the jax-device ->
    physical-tpb mapping never matters.
"""

import numpy as np

S = 32768
H = 1024
NCORES = 8
SL = S // NCORES  # 4096 tokens per core
HC = H // 128  # 8 h-chunks of 128
NGL = SL // 512  # 8 token groups of 512 per core
WR = H + 1  # W_aug row length: [v | W row]
FB = SL // 128  # 32 energy columns per core block

# energy-chunk processing order ~ expected DMA arrival order for the
# dual-queue split (sync: W,q0,q1,q2 / scalar: q3,q4,q5,q6,q7)
CORDER = [3, 4, 0, 5, 1, 6, 2, 7]

DEBUG = True  # extra tiny outputs (u, local energies) for bisection

_cached = {}


def _build():
    """Build + compile the SPMD Bass module (same NEFF on all 8 cores)."""
    from contextlib import ExitStack

    import concourse.bass as bass
    import concourse.mybir as mybir
    import concourse.tile as tile
    from concourse import bacc
    from concourse.masks import make_identity

    f32 = mybir.dt.float32
    i32 = mybir.dt.int32
    AX = mybir.AxisListType
    OP = mybir.AluOpType
    ds = bass.ds

    nc = bacc.Bacc(
        "TRN2", target_bir_lowering=False, debug=False, num_devices=NCORES
    )

    qts = nc.dram_tensor("qts", [128, HC, SL], f32, kind="ExternalInput")
    waug = nc.dram_tensor("waug", [128, HC, WR], f32, kind="ExternalInput")
    out = nc.dram_tensor("out", [SL], f32, kind="ExternalOutput")
    if DEBUG:
        dbg_u = nc.dram_tensor("dbg_u", [1, H], f32, kind="ExternalOutput")
        dbg_e = nc.dram_tensor("dbg_e", [1, SL], f32, kind="ExternalOutput")

    rg = [list(range(NCORES))]
    rsem = nc.alloc_semaphore("rsem")  # peer energy blocks landed (+2 each)
    lsem = nc.alloc_semaphore("lsem")  # our sends on the wire (+16 each)
    psem = nc.alloc_semaphore("psem")  # broadcast descriptors generated

    with tile.TileContext(nc) as tc, ExitStack() as ctx:
        const = ctx.enter_context(tc.tile_pool(name="const", bufs=1))
        qpool = ctx.enter_context(tc.tile_pool(name="qpool", bufs=HC))
        work = ctx.enter_context(tc.tile_pool(name="work", bufs=1))
        psum_e = ctx.enter_context(tc.tile_pool(name="psum_e", bufs=2, space="PSUM"))
        psum_s = ctx.enter_context(tc.tile_pool(name="psum_s", bufs=2, space="PSUM"))
        dram = ctx.enter_context(tc.tile_pool(name="dram", bufs=1, space="DRAM"))

        # re-execution guard: clear the handshake sems BEFORE the entry
        # barrier (a peer can only send after the barrier proves we cleared)
        nc.gpsimd.sem_clear(rsem)
        nc.gpsimd.sem_clear(lsem)
        nc.gpsimd.sem_clear(psem)
        tc.no_sync_barrier()

        # kernel-entry barrier: tiny ncfw AllGather, fired immediately so its
        # ~40-60 us setup floor overlaps the stream; completion => every core
        # is in this kernel with sems cleared, so remote writes are safe.
        # bar_sb is reg_load-ed inside the critical section, so the exchange
        # is fenced behind the barrier by ordinary tile data deps.
        bar_in = dram.tile([1, 1], i32)
        bar_out = dram.tile([NCORES, 1], i32)
        nc.gpsimd.collective_compute(
            "AllGather", OP.bypass, replica_groups=rg,
            ins=[bar_in.opt()], outs=[bar_out.opt()],
        )
        bar_sb = work.tile([1, 1], i32)
        nc.scalar.dma_start(bar_sb[:], bar_out[0:1, :])

        # ---- dual-queue stream, fat descriptors only ----
        w_sb = const.tile([128, HC, WR], f32)
        nc.sync.dma_start(w_sb[:], waug[:])
        q_sb = [None] * HC
        for c in range(HC):
            q = qpool.tile([128, SL], f32, tag="q", name=f"q{c}")
            q_sb[c] = q
        for c in (0, 1, 2):
            nc.sync.dma_start(q_sb[c][:], qts[:, c, :])
        for c in (3, 4, 5, 6, 7):
            nc.scalar.dma_start(q_sb[c][:], qts[:, c, :])

        ident = const.tile([128, 128], f32)
        make_identity(nc, ident[:])
        # pre-warm the Exp table while the stream runs (~1.5 us off the tail)
        warm = work.tile([1, 1], f32)
        nc.scalar.activation(
            warm[:], w_sb[0:1, 0, 0:1], mybir.ActivationFunctionType.Exp
        )

        # ---- u = W^T v: chunk c contracts o = p*8+c over partitions p ----
        pua = psum_s.tile([1, 512], f32, tag="stat")
        pub = psum_s.tile([1, 512], f32, tag="statb")
        for c in range(HC):
            nc.tensor.matmul(
                pua[:], lhsT=w_sb[:, c, 0:1], rhs=w_sb[:, c, 1 : 1 + 512],
                start=(c == 0), stop=(c == HC - 1),
            )
            nc.tensor.matmul(
                pub[:], lhsT=w_sb[:, c, 0:1], rhs=w_sb[:, c, 513 : 513 + 512],
                start=(c == 0), stop=(c == HC - 1),
            )
        u_row = const.tile([1, H], f32)
        nc.vector.tensor_copy(u_row[0:1, 0:512], pua[:])
        nc.scalar.copy(u_row[0:1, 512:1024], pub[:])
        # u_sb[p, c] = u[p*8+c]: transpose strided [1,128] views back to
        # partition-major columns
        u_sb = const.tile([128, HC], f32)
        u_pc = u_row[:].rearrange("one (p c) -> one p c", c=HC)
        for c in range(HC):
            ptc = psum_s.tile([128, 1], f32, tag="stat" if c % 2 == 0 else "statb")
            nc.tensor.transpose(ptc[:], u_pc[:, :, c], ident[0:1, 0:1])
            if c % 2 == 0:
                nc.vector.tensor_copy(u_sb[:, c : c + 1], ptc[:])
            else:
                nc.scalar.copy(u_sb[:, c : c + 1], ptc[:])

        # ---- complete energies for this core's 4096 tokens ----
        # chunk order follows expected DMA arrival (accumulation commutes)
        peA = psum_e.tile([128, 512], f32, tag="peA")
        peB = psum_e.tile([128, 512], f32, tag="peB")
        for ci, c in enumerate(CORDER):
            for g in range(NGL):
                tgt = peA if g < 4 else peB
                j = g % 4
                nc.tensor.matmul(
                    tgt[32 * j : 32 * j + 1, :],
                    lhsT=u_sb[:, c : c + 1],
                    rhs=q_sb[c][:, ds(g * 512, 512)],
                    start=(ci == 0), stop=(ci == HC - 1),
                    tile_position=(0, 32 * j),
                )

        # e_all[p, 32k + f]: slot k = tokens of core self^k; slot 0 = ours.
        # Reshape psum rows {0,32,64,96} (token t = 512g + s) into slot 0
        # (t = 32p + f) with two tiny SBUF->SBUF DMAs.
        e_all = work.tile([128, NCORES * FB], f32)
        esbA = work.tile([128, 512], f32)
        nc.vector.tensor_copy(esbA[:], peA[:])
        esbB = work.tile([128, 512], f32)
        nc.scalar.copy(esbB[:], peB[:])
        rowsA = esbA[:].rearrange("(a b) s -> a b s", b=32)
        rowsB = esbB[:].rearrange("(a b) s -> a b s", b=32)
        dstA = e_all[0:64, 0:FB].rearrange("(g q) f -> g q f", q=16)
        dstB = e_all[64:128, 0:FB].rearrange("(g q) f -> g q f", q=16)
        nc.sync.dma_start(dstA, rowsA[:, 0, :])
        nc.scalar.dma_start(dstB, rowsB[:, 0, :])

        # ---- exchange: 7 direct SDMA writes into the peers' e_all ----
        # slot k goes to core self^k (relative XOR routing, all static).
        # Manual-semaphore handshake: opaque to the tile scheduler's sim.
        with tc.tile_critical():
            # entry gate: reg_load of bar_sb makes the whole section (and
            # with it the trigger) depend on the kernel-entry barrier
            with nc.gpsimd.register() as rbar:
                nc.gpsimd.reg_load(rbar, bar_sb[0:1, 0:1])
            for k in range(1, NCORES):
                rdests = [None] * NCORES
                rdests[k] = (0, k)
                prep = nc.gpsimd.remote_dma_broadcast(
                    e_all[:, FB * k : FB * (k + 1)], e_all[:, 0:FB],
                    remote_sem=rsem, local_sem=lsem, rdests=rdests,
                )
                prep.then_inc(psem, 1)
            nc.gpsimd.wait_ge(psem, NCORES - 1)  # descriptors committed
            nc.gpsimd.trigger_dma(count=NCORES - 1)
            nc.gpsimd.wait_ge(lsem, 16 * (NCORES - 1))  # sends on the wire
            nc.vector.wait_ge(rsem, 2 * (NCORES - 1))  # peer blocks landed
            # marks e_all as an output of the critical section so the
            # softmax below is fenced behind the exchange (value unchanged)
            nc.vector.tensor_copy(e_all[0:1, 0:1], e_all[0:1, 0:1])

        # ---- global softmax over all 32768 energies ----
        # (column blocks are XOR-permuted per core: irrelevant for stats)
        F = NCORES * FB  # 256
        negrow = work.tile([128, 1], f32)
        nc.vector.tensor_reduce(negrow[:], e_all[:], axis=AX.X, op=OP.max, negate=True)
        ex1 = work.tile([128, F], f32)
        rowsum = work.tile([128, 1], f32)
        nc.scalar.activation(
            ex1[:], e_all[:], mybir.ActivationFunctionType.Exp,
            bias=negrow[:], scale=1.0, accum_out=rowsum[:],
        )
        ptr_a = psum_s.tile([1, 128], f32, tag="stat")
        nc.tensor.transpose(ptr_a[:], negrow[:], ident[:])
        ptr_b = psum_s.tile([1, 128], f32, tag="statb")
        nc.tensor.transpose(ptr_b[:], rowsum[:], ident[:])
        tp0 = work.tile([1, 128], f32)
        nc.vector.tensor_copy(tp0[:], ptr_a[:])
        tp1 = work.tile([1, 128], f32)
        nc.scalar.copy(tp1[:], ptr_b[:])
        # global stats on one partition: m = max_j rowmax_j, s = sum_j
        # rowsum_j * exp(rowmax_j - m); tp0 holds -rowmax_j, tp1 rowsum_j
        negm = work.tile([1, 1], f32)
        nc.vector.tensor_reduce(negm[:], tp0[:], axis=AX.X, op=OP.min)
        texp = work.tile([1, 128], f32)
        nc.scalar.activation(
            texp[:], tp0[:], mybir.ActivationFunctionType.Exp,
            bias=negm[:], scale=-1.0,
        )
        prod = work.tile([1, 128], f32)
        nc.vector.tensor_mul(prod[:], texp[:], tp1[:])
        stot = work.tile([1, 1], f32)
        nc.vector.tensor_reduce(stot[:], prod[:], axis=AX.X, op=OP.add)
        rtot = work.tile([1, 1], f32)
        nc.vector.reciprocal(rtot[:], stot[:])
        # K=1 matmul does transpose + scale in one: scl[j] = texp[j] / s
        pscl = psum_s.tile([128, 1], f32, tag="statb")
        nc.tensor.matmul(pscl[:], lhsT=texp[:], rhs=rtot[:], start=True, stop=True)
        scl = work.tile([128, 1], f32)
        nc.vector.tensor_copy(scl[:], pscl[:])
        # only our own tokens (slot 0) go to DRAM; host concatenates ranks
        outt = work.tile([128, FB], f32)
        nc.vector.tensor_scalar_mul(outt[:], ex1[:, 0:FB], scl[:])
        nc.sync.dma_start(out[:].rearrange("(p f) -> p f", f=FB), outt[:])

        if DEBUG:
            nc.scalar.dma_start(dbg_u[:], u_row[:])
            nc.scalar.dma_start(
                dbg_e[:].rearrange("one (p f) -> (one p) f", f=FB), e_all[:, 0:FB]
            )

    nc.compile()
    return nc


def _get_nc():
    if "nc" not in _cached:
        _cached["nc"] = _build()
    return _cached["nc"]


def make_in_maps(question, W, v):
    q = np.ascontiguousarray(np.asarray(question, dtype=np.float32))
    Wn = np.ascontiguousarray(np.asarray(W, dtype=np.float32))
    vn = np.ascontiguousarray(np.asarray(v, dtype=np.float32))
    # W_aug[p, c, 0] = v[p*8+c]; W_aug[p, c, 1+j] = W[p*8+c, j]
    waug = np.empty((128, HC, WR), dtype=np.float32)
    waug[:, :, 0] = vn.reshape(128, HC)
    waug[:, :, 1:] = Wn.reshape(128, HC, H)
    in_maps = []
    for r in range(NCORES):
        # qts[p, c, s] = q[r*SL+s, p*8+c]
        qt = np.ascontiguousarray(q[r * SL : (r + 1) * SL, :].T)  # [H, SL]
        in_maps.append({"qts": qt.reshape(128, HC, SL), "waug": waug})
    return in_maps


def run(question, W, v, **spmd_kwargs):
    """Run the SPMD kernel; returns (out [S] fp32, BassKernelResults)."""
    from concourse.bass_utils import run_bass_kernel_spmd

    nc = _get_nc()
    in_maps = make_in_maps(question, W, v)
    res = run_bass_kernel_spmd(nc, in_maps, core_ids=list(range(NCORES)), **spmd_kwargs)
    full = np.concatenate(
        [np.asarray(res.results[r]["out"], dtype=np.float32) for r in range(NCORES)]
    )
    return full, res


def kernel(question, W, b, v):
    out, _ = run(question, W, v)
    return out.reshape(1, 1, S)
